# revision 11
# baseline (speedup 1.0000x reference)
"""ATKT (LSTM + degenerate causal attention + FC) Trainium2 kernel.

Full inputs in, full outputs out. Internally shards the batch (64) across
8 NeuronCores (8 sequences per core), runs a Bass/Tile kernel per core via
run_bass_kernel_spmd, and reassembles the full [64, 512, 1024] output.

Key algebraic restructurings (exact, not approximations):
 - The per-token embedding lookup + input projection collapses into a
   2048x1024 lookup table T[corr*1024 + cid] built host-side from the
   weights only; the device gathers one 4KB row per token (indirect DMA).
 - The attention scores depend only on the source position j, so the
   causal softmax collapses to running cumulative sums along T
   (tensor_tensor_scan), never materializing BxTxT.
 - Gate order is permuted host-side to [i, f, o, g] so one sigmoid
   activation instruction covers i,f,o and one tanh covers g.
 - All biases are folded into matmuls as rank-1 (ones-vector) terms.
"""
import os
import sys

sys.path.insert(0, "/opt/trn_rl_repo")

import numpy as np
import ml_dtypes

B, T = 64, 512
DC = DR = DL = DA = 256
NC = 1024
N_CORES = 8
BC = B // N_CORES          # sequences per core
TOK = BC * T               # tokens per core (4096)

# ----------------------------------------------------------------------------
# Walrus workaround: this container's neuronxcc rejects >1 sync wait per
# instruction ("Too many sync wait commands"). Split multi-wait instructions
# into single-wait NoOps on the same engine.
# ----------------------------------------------------------------------------


def _apply_tile_patches():
    import bass_rust
    import concourse.tile as tile
    from concourse import mybir

    if getattr(tile.TileContext, "_waitsplit_patched", False):
        return

    _orig_lower = tile.TileContext._lower_ordered_insts

    def _split_waits_in_list(uid, insts, counter):
        new_list = []
        for inst in insts:
            si = inst.sync_info
            if si is not None and len(si.on_wait) > 1:
                waits = list(si.on_wait)
                for w in waits[:-1]:
                    counter[0] += 1
                    nop = mybir.InstNoOp(
                        name=f"waitsplit_{uid}_{counter[0]}",
                        engine=inst.engine,
                        sync_info=bass_rust.SyncInfo(on_wait=[w], on_update=[]),
                        bass_nofuse=True,
                    )
                    new_list.append(nop)
                inst.sync_info = bass_rust.SyncInfo(
                    on_wait=[waits[-1]], on_update=list(si.on_update))
            new_list.append(inst)
        return new_list

    def _patched_lower(self, ordered):
        counter = [0]
        for bb_name in list(ordered.keys()):
            ordered[bb_name] = _split_waits_in_list(self.uid, ordered[bb_name], counter)
        return _orig_lower(self, ordered)

    def _patched_drain_and_barrier(self, tick_clock, wait_clock):
        nc = self.nc
        drain_inst = nc.sync.drain()
        wait_clock.add_sem_waits(
            drain_inst.ins, tile.ScopedClock({None: tick_clock.global_clock}))
        si = drain_inst.ins.sync_info
        if si is not None and len(si.on_wait) > 1:
            waits = list(si.on_wait)
            drain_inst.ins.sync_info = bass_rust.SyncInfo(
                on_wait=waits[:1], on_update=list(si.on_update))
            for w in waits[1:]:
                nop = nc.sync.nop(nofuse=True)
                nop.ins.sync_info = bass_rust.SyncInfo(on_wait=[w], on_update=[])
        nc.all_engine_barrier()
        assert self.sems is not None
        popped = nc._tile_sem_poison_stack.pop()
        assert popped is self._sem_poison
        nc.clear_and_free_semaphores(list(self.sems.allocated().values()))
        nc.all_engine_barrier()

    tile.TileContext._lower_ordered_insts = _patched_lower
    tile.TileContext._drain_and_barrier = _patched_drain_and_barrier
    tile.TileContext._waitsplit_patched = True


# ----------------------------------------------------------------------------
# Kernel build
# ----------------------------------------------------------------------------

def build_kernel(t_steps=T):
    import concourse.bass as bass
    import concourse.tile as tile
    from concourse import mybir

    _apply_tile_patches()

    f32 = mybir.dt.float32
    bf16 = mybir.dt.bfloat16
    i32 = mybir.dt.int32
    AF = mybir.ActivationFunctionType
    OP = mybir.AluOpType

    nc = bass.Bass("TRN2", target_bir_lowering=False, debug=False,
                   num_devices=N_CORES)

    n_tok = BC * t_steps
    n_tc = n_tok // 128            # 128-token chunks
    tc_per_seq = t_steps // 128

    # ---- DRAM parameters (per core) ----
    Tbl = nc.dram_tensor("tbl", [2 * NC, NC], f32, kind="ExternalInput").ap()
    cseq = nc.dram_tensor("cseq", [BC, t_steps], i32, kind="ExternalInput").ap()
    rseq = nc.dram_tensor("rseq", [BC, t_steps], i32, kind="ExternalInput").ap()
    whhT = nc.dram_tensor("whhT", [DL, 4 * DL], f32, kind="ExternalInput").ap()
    mlpWT = nc.dram_tensor("mlpWT", [DL, DA], f32, kind="ExternalInput").ap()
    mlpb = nc.dram_tensor("mlpb", [1, DA], f32, kind="ExternalInput").ap()
    simW = nc.dram_tensor("simW", [DA, BC], f32, kind="ExternalInput").ap()
    fcWT = nc.dram_tensor("fcWT", [2 * DL, NC], f32, kind="ExternalInput").ap()
    fcb = nc.dram_tensor("fcb", [1, NC], f32, kind="ExternalInput").ap()
    yout = nc.dram_tensor("y", [n_tok, NC], f32, kind="ExternalOutput").ap()

    with tile.TileContext(nc) as tc:
        import contextlib
        with contextlib.ExitStack() as ctx:
            g_pool = ctx.enter_context(tc.tile_pool(name="globals", bufs=1))
            lstm_pool = ctx.enter_context(tc.tile_pool(name="lstm", bufs=1))

            # ---- persistent small tiles ----
            ones = g_pool.tile([128, 512], f32)
            nc.vector.memset(ones, 1.0)
            ident = g_pool.tile([128, 128], f32)
            nc.vector.memset(ident, 1.0)
            nc.gpsimd.affine_select(
                out=ident, in_=ident, pattern=[[-1, 128]],
                compare_op=OP.is_equal, fill=0.0, base=0, channel_multiplier=1)

            whh_sb = g_pool.tile([128, 2, 4 * DL], f32)
            nc.sync.dma_start(
                out=whh_sb,
                in_=whhT.rearrange("(k p) g -> p k g", p=128))

            h_bf = g_pool.tile([128, 2, BC], f32)
            c_fp = g_pool.tile([128, 2, BC], f32)
            nc.vector.memset(h_bf, 0.0)
            nc.vector.memset(c_fp, 0.0)

            # lstm_out feature-major: [p, k(2 H-chunks), b, t]
            lstm_fm = lstm_pool.tile([128, 2, BC, t_steps], f32)

            # ================= Phase 1: gather + transpose xg ==============
            with tc.tile_pool(name="xg", bufs=1) as xg_pool, \
                 tc.tile_pool(name="p1tmp", bufs=3) as p1_pool, \
                 tc.tile_pool(name="p1psum", bufs=2, space="PSUM") as p1_psum:

                # offsets: idx = corr*1024 + cid, laid out [p=tok%128, chunk]
                cid32 = p1_pool.tile([n_tc, 128], f32, tag="cid")
                rid32 = p1_pool.tile([n_tc, 128], f32, tag="rid")
                # DRAM [BC, t] viewed as [(BC*tc_per_seq), 128] row-chunks
                nc.gpsimd.dma_start(out=cid32,
                                    in_=cseq.rearrange("b (c p) -> (b c) p", p=128))
                nc.gpsimd.dma_start(out=rid32,
                                    in_=rseq.rearrange("b (c p) -> (b c) p", p=128))
                idxf = p1_pool.tile([n_tc, 128], f32, tag="idxf")
                nc.vector.tensor_scalar_mul(idxf, rid32, float(NC))
                nc.vector.tensor_add(idxf, idxf, cid32)
                idx_ps = p1_psum.tile([128, n_tc], f32, tag="idxps")
                nc.tensor.transpose(out=idx_ps, in_=idxf, identity=ident[:n_tc, :n_tc])
                offs = g_pool.tile([128, n_tc], i32)
                nc.vector.tensor_copy(out=offs, in_=idx_ps)

                xg_fm = xg_pool.tile([128, 8, BC, t_steps], f32)

                for c in range(n_tc):
                    row = p1_pool.tile([128, NC], f32, tag="gath")
                    nc.gpsimd.indirect_dma_start(
                        out=row, out_offset=None,
                        in_=Tbl,
                        in_offset=bass.IndirectOffsetOnAxis(ap=offs[:, c:c + 1], axis=0),
                    )
                    b = c // tc_per_seq
                    t0 = (c % tc_per_seq) * 128
                    for j in range(8):
                        tp = p1_psum.tile([128, 128], f32, tag="tp")
                        nc.tensor.transpose(
                            out=tp, in_=row[:, 128 * j:128 * (j + 1)], identity=ident)
                        eng = nc.scalar if (j % 2 == 0) else nc.vector
                        if j % 2 == 0:
                            nc.scalar.copy(out=xg_fm[:, j, b, t0:t0 + 128], in_=tp)
                        else:
                            nc.vector.tensor_copy(out=xg_fm[:, j, b, t0:t0 + 128], in_=tp)

                # ================= Phase 2: LSTM recurrence ================
                with tc.tile_pool(name="rec_ps", bufs=2, space="PSUM") as rec_psum, \
                     tc.tile_pool(name="rec_sb", bufs=3) as rec_pool:
                    for t in range(t_steps):
                        gates_ps = rec_psum.tile([128, 8, BC], f32, tag="gps")
                        for j in range(8):
                            for k in range(2):
                                nc.tensor.matmul(
                                    out=gates_ps[:, j, :],
                                    lhsT=whh_sb[:, k, 128 * j:128 * (j + 1)],
                                    rhs=h_bf[:, k, :],
                                    start=(k == 0), stop=(k == 1))
                        gsb = rec_pool.tile([128, 8, BC], f32, tag="gsb")
                        nc.vector.tensor_add(gsb, gates_ps, xg_fm[:, :, :, t])
                        act = rec_pool.tile([128, 8, BC], f32, tag="act")
                        gflat = gsb.rearrange("p j b -> p (j b)")
                        aflat = act.rearrange("p j b -> p (j b)")
                        nc.scalar.activation(
                            out=aflat[:, 0:6 * BC], in_=gflat[:, 0:6 * BC], func=AF.Sigmoid)
                        nc.scalar.activation(
                            out=aflat[:, 6 * BC:8 * BC], in_=gflat[:, 6 * BC:8 * BC],
                            func=AF.Tanh)
                        i_ap = act[:, 0:2, :].rearrange("p j b -> p (j b)")
                        f_ap = act[:, 2:4, :].rearrange("p j b -> p (j b)")
                        o_ap = act[:, 4:6, :].rearrange("p j b -> p (j b)")
                        g_ap = act[:, 6:8, :].rearrange("p j b -> p (j b)")
                        cflat = c_fp.rearrange("p k b -> p (k b)")
                        ig = rec_pool.tile([128, 2 * BC], f32, tag="ig")
                        nc.vector.tensor_mul(ig, i_ap, g_ap)
                        fcp = rec_pool.tile([128, 2 * BC], f32, tag="fcp")
                        nc.vector.tensor_mul(fcp, f_ap, cflat)
                        nc.vector.tensor_add(cflat, fcp, ig)
                        tc_t = rec_pool.tile([128, 2 * BC], f32, tag="tct")
                        nc.scalar.activation(out=tc_t, in_=cflat, func=AF.Tanh)
                        hslot = lstm_fm[:, :, :, t].rearrange("p k b -> p (k b)")
                        nc.vector.tensor_mul(hslot, o_ap, tc_t)
                        nc.vector.tensor_copy(
                            out=h_bf.rearrange("p k b -> p (k b)"), in_=hslot)

            # ================= Phase 3: attention + FC =====================
            with tc.tile_pool(name="p3", bufs=1) as p3_pool, \
                 tc.tile_pool(name="p3att", bufs=3) as p3a_pool, \
                 tc.tile_pool(name="p3tmp", bufs=1) as p3t_pool, \
                 tc.tile_pool(name="p3out", bufs=3) as p3o_pool, \
                 tc.tile_pool(name="p3ps_a", bufs=2, space="PSUM") as p3_psum_a, \
                 tc.tile_pool(name="p3ps_s", bufs=2, space="PSUM") as p3_psum_s, \
                 tc.tile_pool(name="p3ps_o", bufs=2, space="PSUM") as p3_psum_o:

                mlp_sb = p3_pool.tile([128, 2, DA], f32)
                nc.sync.dma_start(out=mlp_sb,
                                  in_=mlpWT.rearrange("(k p) a -> p k a", p=128))
                mlpb_sb = p3_pool.tile([1, DA], f32)
                nc.sync.dma_start(out=mlpb_sb, in_=mlpb)
                sim_sb = p3_pool.tile([128, 2, BC], f32)
                nc.sync.dma_start(out=sim_sb,
                                  in_=simW.rearrange("(k p) o -> p k o", p=128))

                lstm_flat = lstm_fm.rearrange("p k b t -> p k (b t)")

                # --- att = tanh(mlp_W @ h + mlp_b); score = sim_W @ att;
                #     w = exp(score). One 512-token chunk == one sequence b.
                w1 = p3t_pool.tile([1, BC, t_steps], f32, tag="wh")
                for b in range(BC):
                    att_n = p3a_pool.tile([128, 2, t_steps], f32, tag="attn_mlp")
                    for m in range(2):
                        aps = p3_psum_a.tile([128, 512], f32, tag="aps")
                        for k in range(2):
                            nc.tensor.matmul(
                                out=aps[:, :t_steps],
                                lhsT=mlp_sb[:, k, 128 * m:128 * (m + 1)],
                                rhs=lstm_fm[:, k, b, :],
                                start=(k == 0), stop=False)
                        nc.tensor.matmul(
                            out=aps[:, :t_steps],
                            lhsT=mlpb_sb[:, 128 * m:128 * (m + 1)],
                            rhs=ones[0:1, :t_steps], start=False, stop=True)
                        nc.scalar.activation(
                            out=att_n[:, m, :], in_=aps[:, :t_steps], func=AF.Tanh)
                    sps = p3_psum_s.tile([1, 512], f32, tag="sps")
                    for m in range(2):
                        nc.tensor.matmul(
                            out=sps[:, :t_steps], lhsT=sim_sb[:, m, 0:1],
                            rhs=att_n[:, m, :],
                            start=(m == 0), stop=(m == 1))
                    nc.scalar.activation(
                        out=w1[0:1, b, :], in_=sps[:, :t_steps], func=AF.Exp)

                # --- cumulative attention mass and its reciprocal ---
                cw1 = p3t_pool.tile([1, BC, t_steps], f32, tag="cum")
                for b in range(BC):
                    nc.vector.tensor_tensor_scan(
                        out=cw1[0:1, b, :], data0=ones[0:1, :t_steps],
                        data1=w1[0:1, b, :],
                        initial=0.0, op0=OP.mult, op1=OP.add)
                rw1 = p3t_pool.tile([1, BC, t_steps], f32, tag="rw")
                nc.vector.reciprocal(
                    out=rw1.rearrange("o b t -> o (b t)"),
                    in_=cw1.rearrange("o b t -> o (b t)"))
                # broadcast w/rw rows across all 128 partitions via rank-1
                # matmuls (ones[128] x row).
                wrep = p3_pool.tile([128, BC, t_steps], f32)
                rwrep = p3_pool.tile([128, BC, t_steps], f32)
                for b in range(BC):
                    for (srcrow, dst) in ((w1, wrep), (rw1, rwrep)):
                        bps = p3_psum_a.tile([128, 512], f32, tag="aps")
                        nc.tensor.matmul(
                            out=bps[:, :t_steps], lhsT=ones[0:1, 0:128],
                            rhs=srcrow[0:1, b, :], start=True, stop=True)
                        nc.scalar.copy(out=dst[:, b, :], in_=bps[:, :t_steps])
                wrep_f = wrep.rearrange("p b t -> p (b t)")
                rwrep_f = rwrep.rearrange("p b t -> p (b t)")

                # --- running weighted mean + exclusive cumsum ---
                excl_fm = p3_pool.tile([128, 2, BC, t_steps], f32)
                for k in range(2):
                    wh = p3t_pool.tile([128, n_tok], f32, tag="wh")
                    nc.vector.tensor_mul(wh, wrep_f, lstm_flat[:, k, :])
                    cum = p3t_pool.tile([128, n_tok], f32, tag="cum")
                    for b in range(BC):
                        nc.vector.tensor_tensor_scan(
                            out=cum[:, b * t_steps:(b + 1) * t_steps],
                            data0=ones[:, :t_steps],
                            data1=wh[:, b * t_steps:(b + 1) * t_steps],
                            initial=0.0, op0=OP.mult, op1=OP.add)
                    attn = wh  # wh is dead; reuse its space for attn_out
                    nc.vector.tensor_mul(attn, cum, rwrep_f)
                    nc.vector.memset(excl_fm[:, k, :, 0], 0.0)
                    for b in range(BC):
                        nc.vector.tensor_tensor_scan(
                            out=excl_fm[:, k, b, 1:t_steps],
                            data0=ones[:, :t_steps - 1],
                            data1=attn[:, b * t_steps:(b + 1) * t_steps - 1],
                            initial=0.0, op0=OP.mult, op1=OP.add)

                # --- FC + sigmoid + output DMA (token-major) ---
                fc_sb = p3_pool.tile([128, 4, NC], f32)
                nc.sync.dma_start(out=fc_sb,
                                  in_=fcWT.rearrange("(k p) c -> p k c", p=128))
                fcb_sb = p3_pool.tile([1, NC], f32)
                nc.sync.dma_start(out=fcb_sb, in_=fcb)

                excl_flat = excl_fm.rearrange("p k b t -> p k (b t)")
                kchunks = [excl_flat[:, 0, :], excl_flat[:, 1, :],
                           lstm_flat[:, 0, :], lstm_flat[:, 1, :]]
                for m in range(n_tc):
                    ops = p3_psum_o.tile([128, NC], f32, tag="ops")
                    for half in range(2):
                        osl = ops[:, 512 * half:512 * (half + 1)]
                        for k in range(4):
                            nc.tensor.matmul(
                                out=osl,
                                lhsT=kchunks[k][:, 128 * m:128 * (m + 1)],
                                rhs=fc_sb[:, k, 512 * half:512 * (half + 1)],
                                start=(k == 0), stop=False)
                        nc.tensor.matmul(
                            out=osl, lhsT=ones[0:1, 0:128],
                            rhs=fcb_sb[:, 512 * half:512 * (half + 1)],
                            start=False, stop=True)
                    osb = p3o_pool.tile([128, NC], f32, tag="osb")
                    nc.scalar.activation(out=osb, in_=ops, func=AF.Sigmoid)
                    nc.sync.dma_start(out=yout[128 * m:128 * (m + 1), :], in_=osb)

    return nc


# ----------------------------------------------------------------------------
# Host-side weight preparation
# ----------------------------------------------------------------------------

def _prepare(inputs):
    W_ih = inputs["W_ih"].astype(np.float64)
    W_hh = inputs["W_hh"].astype(np.float64)
    b_ih = inputs["b_ih"].astype(np.float64)
    b_hh = inputs["b_hh"].astype(np.float64)
    ec = inputs["embed_concept"].astype(np.float64)
    er = inputs["embed_correct"].astype(np.float64)

    W_A = W_ih[:, :DC]
    W_B = W_ih[:, DC:]
    bias = b_ih + b_hh
    # T[0*NC + cid] : corr=0 -> inter=[v0; u]  => W_A v0 + W_B u + bias
    # T[1*NC + cid] : corr=1 -> inter=[u; v1]  => W_A u + W_B v1 + bias
    T0 = ec @ W_B.T + (W_A @ er[0] + bias)[None, :]
    T1 = ec @ W_A.T + (W_B @ er[1] + bias)[None, :]
    Tbl = np.concatenate([T0, T1], axis=0)

    # gate permutation [i, f, g, o] -> [i, f, o, g]
    perm = np.concatenate([np.arange(0, 2 * DL),
                           np.arange(3 * DL, 4 * DL),
                           np.arange(2 * DL, 3 * DL)])
    Tbl = Tbl[:, perm].astype(np.float32)
    whhT = np.ascontiguousarray(W_hh[perm].T).astype(np.float32)

    return {
        "tbl": np.ascontiguousarray(Tbl),
        "whhT": np.ascontiguousarray(whhT),
        "mlpWT": np.ascontiguousarray(inputs["mlp_W"].T.astype(np.float32)),
        "mlpb": np.ascontiguousarray(inputs["mlp_b"].astype(np.float32)[None, :]),
        "simW": np.ascontiguousarray(np.tile(inputs["sim_W"].astype(np.float32).reshape(DA, 1), (1, BC))),
        "fcWT": np.ascontiguousarray(inputs["fc_W"].T.astype(np.float32)),
        "fcb": np.ascontiguousarray(inputs["fc_b"].astype(np.float32)[None, :]),
    }


_CACHE = {}


def kernel(**inputs):
    from concourse.bass_utils import run_bass_kernel_spmd

    if "nc" not in _CACHE:
        _CACHE["nc"] = build_kernel()
    nc = _CACHE["nc"]

    shared = _prepare(inputs)
    cseq = np.ascontiguousarray(inputs["concept_seq"].astype(np.int32))
    rseq = np.ascontiguousarray(inputs["correct_seq"].astype(np.int32))

    in_maps = []
    for i in range(N_CORES):
        m = dict(shared)
        m["cseq"] = np.ascontiguousarray(cseq[i * BC:(i + 1) * BC])
        m["rseq"] = np.ascontiguousarray(rseq[i * BC:(i + 1) * BC])
        in_maps.append(m)

    res = run_bass_kernel_spmd(nc, in_maps, list(range(N_CORES)))
    out = np.concatenate(
        [res.results[i]["y"].reshape(BC, T, NC) for i in range(N_CORES)], axis=0)
    return out.astype(np.float32)
